# revision 4
# baseline (speedup 1.0000x reference)
"""Per-class variance penalty (segment-reduce) on 8 TRN2 NeuronCores.

Strategy (data-parallel over N): each core streams its 1/8 shard of x
through the TensorEngine as ``stats += onehot(t)^T @ [x | x^2]``,
accumulating per-class sums and sums-of-squares for all 100 classes in a
single PSUM bank across 256 row-tiles.  The one-hot is built on-chip by
comparing an iota row against the class id (per-partition scalar).  The
8 partial [C, 2D] statistics are summed on the host, where the final
(tiny) [C, D] variance/L1 reduction is done in fp64-free numpy.

Data is shipped as bf16 (exactly representable one-hot weights, bf16
x / x^2 streams, fp32 PSUM accumulation).  The output is a single scalar
averaged over C*D = 25.6k statistics, so the bf16 rounding noise averages
out ~1e-4 relative — far inside tolerance — while halving HBM traffic.
"""

import numpy as np
import ml_dtypes

import concourse.bass as bass
import concourse.tile as tile
from concourse import bacc, mybir
from concourse.bass_utils import run_bass_kernel_spmd

N_CORES = 8
N, D, C = 262144, 256, 100
N_SHARD = N // N_CORES          # 32768 rows per core
P = 128                          # SBUF partitions / PE contraction dim
N_TILES = N_SHARD // P           # 256 row-tiles per core
BF16 = mybir.dt.bfloat16
FP32 = mybir.dt.float32

_compiled = None


def _build():
    nc = bacc.Bacc("TRN2", target_bir_lowering=False, debug=False,
                   num_devices=N_CORES)
    x_d = nc.dram_tensor("x", [N_SHARD, D], BF16, kind="ExternalInput").ap()
    t_d = nc.dram_tensor("t", [P, N_TILES], FP32, kind="ExternalInput").ap()
    iota_d = nc.dram_tensor("iota", [P, P], BF16, kind="ExternalInput").ap()
    stats_d = nc.dram_tensor("stats", [P, 2 * D], FP32,
                             kind="ExternalOutput").ap()

    with tile.TileContext(nc) as tc:
        with (
            tc.tile_pool(name="const", bufs=1) as const_pool,
            tc.tile_pool(name="x", bufs=8) as x_pool,
            tc.tile_pool(name="oh", bufs=6) as oh_pool,
            tc.tile_pool(name="psum", bufs=1, space=bass.MemorySpace.PSUM) as psum_pool,
        ):
            tsb = const_pool.tile([P, N_TILES], FP32, tag="tsb")
            nc.sync.dma_start(tsb[:], t_d[:])
            iota = const_pool.tile([P, P], BF16, tag="iota")
            nc.sync.dma_start(iota[:], iota_d[:])

            acc = psum_pool.tile([P, 2 * D], FP32)

            for i in range(N_TILES):
                xt = x_pool.tile([P, 2 * D], BF16)
                nc.sync.dma_start(xt[:, 0:D], x_d[i * P:(i + 1) * P, :])
                nc.scalar.activation(xt[:, D:2 * D], xt[:, 0:D],
                                     mybir.ActivationFunctionType.Square)
                oh = oh_pool.tile([P, P], BF16)
                nc.vector.tensor_scalar(oh[:], iota[:], tsb[:, i:i + 1], None,
                                        mybir.AluOpType.is_equal)
                nc.tensor.matmul(acc[:], oh[:], xt[:],
                                 start=(i == 0), stop=(i == N_TILES - 1))

            out_sb = const_pool.tile([P, 2 * D], FP32, tag="out_sb")
            nc.vector.tensor_copy(out_sb[:], acc[:])
            nc.sync.dma_start(stats_d[:], out_sb[:])

    nc.compile()
    return nc


def _prepare_in_maps(x: np.ndarray, t: np.ndarray) -> list[dict]:
    xh = np.asarray(x).astype(ml_dtypes.bfloat16)
    t = np.asarray(t)
    iota = np.broadcast_to(np.arange(P, dtype=np.float32), (P, P)).astype(
        ml_dtypes.bfloat16)
    in_maps = []
    for c in range(N_CORES):
        xs = xh[c * N_SHARD:(c + 1) * N_SHARD]
        ts = t[c * N_SHARD:(c + 1) * N_SHARD]
        # tsb[p, i] = class id of row i*P + p of this shard
        tsb = np.ascontiguousarray(
            ts.reshape(N_TILES, P).T.astype(np.float32))
        in_maps.append({"x": xs, "t": tsb, "iota": iota})
    return in_maps


def kernel(x: np.ndarray, t: np.ndarray) -> np.ndarray:
    global _compiled
    if _compiled is None:
        _compiled = _build()
    nc = _compiled

    t = np.asarray(t)
    in_maps = _prepare_in_maps(x, t)
    res = run_bass_kernel_spmd(nc, in_maps, list(range(N_CORES)))

    s = np.zeros((C, D), np.float32)
    sq = np.zeros((C, D), np.float32)
    for c in range(N_CORES):
        stats = res.results[c]["stats"]
        s += stats[:C, 0:D]
        sq += stats[:C, D:2 * D]

    cnt = np.bincount(t.astype(np.int64), minlength=C).astype(np.float32)
    n = cnt[:, None]
    var = (sq - s * s / n) / (n - 1.0)
    penalty = np.abs(var).sum(dtype=np.float32) / np.float32(C)
    return np.asarray(penalty, dtype=np.float32).reshape(1)
